# revision 21
# baseline (speedup 1.0000x reference)
"""Trainium2 Bass kernel for local sparse attention (k=16 neighbors).

Reference computation (b=4, n=8192, k=16, d=128):
    Q = src @ Wq.T ; K = tgt @ Wk.T ; V = tgt @ Wv.T
    scores = einsum('bnkd,bnd->bnk', K, Q) / sqrt(d)
    out = einsum('bnk,bnkd->bnd', softmax(scores), V)

Restructured so the 34-GFLOP K/V projections are never materialized:
    scores[n,k] = tgt[n,k,:] . qw[n,:],  qw = src @ (Wq.T Wk / sqrt(d))
    out[n,:]    = (sum_k e[n,k] * tgt[n,k,:]) @ Wv.T / sum_k e[n,k]

Device pipeline per 128-point tile (all tensors fp16, fp32 accumulation):
  1. SEGDOT_ANT   (custom DVE op, 2x_1p fp16 mode, hand-written uops):
     per-page dot products via a segmented multiply-cumsum whose
     accumulator RESETS at each 128-element page boundary; page-end
     positions hold scores[p, k]. Two fp16 MACs per lane-cycle.
  2. ScalarE exp on the 16 page-end values (read twice each ->
     duplicated pairs e2[p,k,2]).
  3. PAGESCALE_ANT (custom DVE op, 2x_1p): scaled[p,k,:] = tn[p,k,:] *
     e[p,k], with e held in swap flops and re-latched per page (src1 is
     consumed as one 4-byte pair per page -> e duplicated pairs).
  4. TensorE: 16 accumulating identity-matmuls produce
     ctxT[d, p] = sum_k scaled[p,k,d] directly in PSUM (transpose and
     reduction in one), then out_raw = ctxT.T @ Wv.T via one more matmul.
  5. fp16 out_raw + fp16 e2 DMA'd out; host computes den = sum_k e and
     divides.

Host does the tiny O(1/32 of flops) parts: qw projection, fp16 casts,
final divide. Sharding: data-parallel over (b*n) across 8 cores.
"""

import math

import numpy as np

B, N_SEQ, KNBR, D = 4, 8192, 16, 128
NCORES = 8
PTS_TOTAL = B * N_SEQ            # 32768
PTS_CORE = PTS_TOTAL // NCORES   # 4096
TILE = 128
NTILES = PTS_CORE // TILE        # 32

_cached = {}

# --------------------------------------------------------------------------
# Custom DVE ops (hand-written uop programs, 1x + 2x_1p variants)
# --------------------------------------------------------------------------


def _register_dve_ops():
    from concourse.dve_ops import (
        DveOp, OPS, CUSTOM_DVE_SPECS, _SUB_OPCODE_FOR_NAME,
        _CUSTOM_DVE_ROW_BASE,
    )
    have = {op.name: op for op in OPS}
    if "SEGDOT_ANT" in have:
        return have["SEGDOT_ANT"], have["PAGESCALE_ANT"]

    from dataclasses import dataclass
    from concourse.dve_spec import (
        Spec, Src0, Src1, C3, scan, AluOp, _spill_c3_to_src1,
    )
    from concourse.dve_uop import (
        DveOpSpec, UopConfig, AluInp, InpSel, OutSel, OutPath,
        Trigger, DelayInp, ENABLE,
    )

    Dl = [AluInp.PREV_DELAY_0, AluInp.PREV_DELAY_1, AluInp.PREV_DELAY_2,
          AluInp.PREV_DELAY_3, AluInp.PREV_DELAY_4, AluInp.PREV_DELAY_5]
    PREV, CURR, SWAP = (AluInp.PREV_ALU_OUT, AluInp.CURR_ALU_OUT,
                        AluInp.CURR_SWAP_OUT)
    T_STEADY = (Trigger.SRC_TENSOR_DONE, Trigger.SUB_DIM_DONE, Trigger.NONE)
    T_STEP = (Trigger.SRC_TENSOR_DONE, Trigger.SUB_DIM_DONE, Trigger.COUNT)
    T_SEED = (Trigger.COUNT, Trigger.NONE, Trigger.NONE)

    def mk(inputs, lanes):
        u = UopConfig()
        for lane_idx, sel in inputs:
            u.enable_input(sel, lane_idx + 1)
        for b in range(8):
            u.datapath_config[b].pass_through_alu()
            u.datapath_config[b].pass_through_delay(*lanes)
        return u

    def seed(u):
        u.trigger, u.repeat_count, u.next_uop = T_SEED, 1, (1, 0, 0)
        return u

    def steady(u, write=True):
        u.require_inp0 = u.require_inp1 = 1
        if write:
            u.enable_output(OutSel.ALU_OUT, OutPath.WR0_LO)
        u.trigger, u.next_uop = T_STEADY, (0, 2, 0)
        return u

    def step(u, write=True):
        u.require_inp0 = u.require_inp1 = 1
        if write:
            u.enable_output(OutSel.ALU_OUT, OutPath.WR0_LO)
        u.trigger, u.next_uop, u.repeat_count = T_STEP, (0, 2, 1), 1
        return u

    # ---- SEGDOT: reset-cumsum of in0*in1 over pages -----------------------
    def segdot_1x():
        INP = [(0, InpSel.SRC_0), (1, InpSel.SRC_1), (2, InpSel.ZERO)]

        def base():
            u = mk(INP, (0, 1, 2))
            u.datapath_config[0].enable_alu(AluOp.MULTIPLY, Dl[0], Dl[1])
            u.datapath_config[1].enable_alu(AluOp.ADD, CURR, PREV)
            return u

        s0 = base()
        s0.datapath_config[1].enable_alu(AluOp.BYPASS, Dl[2])   # acc <- 0
        s1 = steady(base())
        s2 = base()
        s2.datapath_config[1].enable_alu(AluOp.BYPASS, PREV)    # acc <- m
        return [seed(s0), s1, step(s2)]

    def segdot_2x():
        INP = [(0, InpSel.SRC_0), (1, InpSel.SRC_1),
               (2, InpSel.SRC_0_HI), (3, InpSel.SRC_1_HI), (4, InpSel.ZERO)]

        def base():
            u = mk(INP, (0, 1, 2, 3, 4))
            dp = u.datapath_config
            dp[0].enable_alu(AluOp.MULTIPLY, Dl[0], Dl[1])           # m0
            dp[1].enable_alu(AluOp.MULTIPLY, Dl[2], Dl[3])           # m1
            dp[1].enable_delay_from_src(DelayInp.PREV_ALU_OUT, 5)    # ch5<-m0
            for b in range(2, 8):
                dp[b].pass_through_delay(5)
            dp[2].enable_alu(AluOp.ADD, PREV, Dl[5])                 # t=m0+m1
            dp[3].enable_alu(AluOp.ADD, CURR, PREV)                  # acc+=t
            return u

        s0 = base()
        s0.datapath_config[3].enable_alu(AluOp.BYPASS, Dl[4])        # acc<-0
        s1 = steady(base())
        s1.enable_output(OutSel.ALU_OUT, OutPath.WR0_HI)  # pair-cumsum both halves
        s2 = base()
        s2.datapath_config[3].enable_alu(AluOp.BYPASS, PREV)         # acc<-t
        step(s2)
        s2.enable_output(OutSel.ALU_OUT, OutPath.WR0_HI)
        return [seed(s0), s1, s2]

    # ---- PAGESCALE: in0 * e[page], e latched per page from src1 ----------
    def pagescale_1x():
        INP = [(0, InpSel.SRC_0), (1, InpSel.SRC_1)]
        init = mk(INP, (0, 1))
        init.datapath_config[0].enable_alu(AluOp.BYPASS, Dl[1])
        init.datapath_config[0].swap_enable = ENABLE
        init.require_inp1 = 1
        seed(init)

        st = mk(INP, (0, 1))
        st.datapath_config[0].enable_alu(AluOp.MULTIPLY, Dl[0], SWAP)
        st.require_inp0 = 1
        st.enable_output(OutSel.ALU_OUT, OutPath.WR0_LO)
        st.trigger, st.next_uop = T_STEADY, (0, 2, 0)

        sp = mk(INP, (0, 1))
        sp.datapath_config[0].enable_alu(AluOp.BYPASS, Dl[1])
        sp.datapath_config[0].swap_enable = ENABLE
        sp.datapath_config[1].enable_alu(AluOp.MULTIPLY, Dl[0], PREV)
        step(sp)
        return [init, st, sp]

    def pagescale_2x():
        INP = [(0, InpSel.SRC_0), (1, InpSel.SRC_0_HI), (2, InpSel.SRC_1)]
        init = mk(INP, (0, 1, 2))
        init.datapath_config[0].enable_alu(AluOp.BYPASS, Dl[2])
        init.datapath_config[0].swap_enable = ENABLE
        init.datapath_config[1].enable_alu(AluOp.BYPASS, PREV)
        init.datapath_config[1].swap_enable = ENABLE
        init.require_inp1 = 1
        seed(init)

        st = mk(INP, (0, 1, 2))
        dp = st.datapath_config
        dp[0].enable_alu(AluOp.MULTIPLY, Dl[0], SWAP)                # r0
        dp[1].enable_alu(AluOp.MULTIPLY, Dl[1], SWAP)                # r1
        dp[1].enable_delay_from_src(DelayInp.PREV_ALU_OUT, 3)        # ch3<-r0
        for b in range(2, 8):
            dp[b].pass_through_delay(3)
        st.require_inp0 = 1
        st.enable_output(OutSel.DELAY_3, OutPath.WR0_LO)             # r0 even
        st.enable_output(OutSel.ALU_OUT, OutPath.WR0_HI)             # r1 odd
        st.trigger, st.next_uop = T_STEADY, (0, 2, 0)

        sp = mk(INP, (0, 1, 2))
        dp = sp.datapath_config
        dp[0].enable_alu(AluOp.BYPASS, Dl[2])                        # e_new
        dp[0].swap_enable = ENABLE
        dp[1].enable_alu(AluOp.BYPASS, PREV)
        dp[1].swap_enable = ENABLE
        dp[2].enable_alu(AluOp.MULTIPLY, Dl[0], PREV)                # r0
        dp[2].enable_delay_from_src(DelayInp.PREV_ALU_OUT, 3)        # ch3<-e
        dp[3].enable_alu(AluOp.MULTIPLY, Dl[1], Dl[3])               # r1
        dp[3].enable_delay_from_src(DelayInp.PREV_ALU_OUT, 4)        # ch4<-r0
        for b in range(3, 8):
            dp[b].pass_through_delay(3)
        for b in range(4, 8):
            dp[b].pass_through_delay(4)
        sp.require_inp0 = sp.require_inp1 = 1
        sp.enable_output(OutSel.DELAY_4, OutPath.WR0_LO)
        sp.enable_output(OutSel.ALU_OUT, OutPath.WR0_HI)
        sp.trigger, sp.next_uop, sp.repeat_count = T_STEP, (0, 2, 1), 1
        return [init, st, sp]

    @dataclass(frozen=True)
    class HandDveOp(DveOp):
        raw_v3: "DveOpSpec | None" = None

        def compile(self, ver):
            assert ver == "v3", f"hand-built op only has v3 uops, got {ver}"
            return self.raw_v3

    def segdot_ref(in0, in1, s0, s1, imm2):
        P, N = in0.shape[0], in0.shape[-1]
        S = int(np.prod(in0.shape[1:-1]))
        a = in0.reshape(P, S, N).astype(np.float32)
        b = np.asarray(in1, np.float32)
        b = (np.broadcast_to(b.reshape(P, 1, N), a.shape)
             if b.size == P * N else b.reshape(a.shape))
        return np.cumsum(a * b, axis=-1).reshape(in0.shape)

    def pagescale_ref(in0, in1, s0, s1, imm2):
        P, N = in0.shape[0], in0.shape[-1]
        S = int(np.prod(in0.shape[1:-1]))
        a = in0.reshape(P, S, N).astype(np.float32)
        e = np.asarray(in1, np.float32).reshape(P, -1)[:, ::2][:, :S]
        return (a * e[:, :, None]).reshape(in0.shape)

    ops = []
    for name, spec, u1, u2 in (
        ("SEGDOT_ANT",
         Spec(body=scan(AluOp.ADD, Src0 * Src1), reference=segdot_ref),
         segdot_1x(), segdot_2x()),
        ("PAGESCALE_ANT",
         Spec(body=Src0 * _spill_c3_to_src1(C3), reference=pagescale_ref),
         pagescale_1x(), pagescale_2x()),
    ):
        row = _CUSTOM_DVE_ROW_BASE + len(OPS)
        raw = DveOpSpec(name=name, opcode=row, uops=u1, uops_2x=u2,
                        perf_max=1, rd1_en=True)
        raw.validate("v3")
        op = HandDveOp(name=name, spec=spec, subdim=True,
                       uops_sha={"v3": raw.sha("v3")}, raw_v3=raw)
        OPS.append(op)
        _SUB_OPCODE_FOR_NAME[name] = row
        CUSTOM_DVE_SPECS[name] = spec
        ops.append(op)
    return ops[0], ops[1]


def _emit_custom(nc, op, *, out, in0, in1, perf_max=1):
    """Like nc.vector._custom_dve but with an explicit perf_max."""
    from concourse import bass_isa, mybir
    from concourse.dve_ops import get_dve_sub_opcode

    vec = nc.vector
    m = vec.bass.m
    if op.name not in m.ant_custom_dve_ops:
        m.ant_custom_dve_ops = sorted({*m.ant_custom_dve_ops, op.name})
    opt = not op.subdim
    in1_elementwise = len(in1.shape) > 2
    shape = (bass_isa.CustomDveShape.STT if in1_elementwise
             else bass_isa.CustomDveShape.TTSS)
    isa_opcode = vec.bass.isa.Opcode[
        f"NEURON_ISA_TPB_OPCODE_CUSTOM_DVE_ANT_{shape.slot()}"
    ].value
    zero = mybir.ImmediateValue(dtype=mybir.dt.float32, value=0.0)
    ins = [vec.lower_ap(in0, for_isa=True, opt=opt),
           vec.lower_ap(in1, for_isa=True, opt=opt), zero, zero]
    outs = [vec.lower_ap(out, for_isa=True, opt=opt)]
    return vec.add_instruction(bass_isa.InstCustomDveAnt(
        name=vec.bass.get_next_instruction_name(),
        op_name=op.name, rd1_en=True, subdim=0x02, imm2=0.0,
        shape=shape, row=get_dve_sub_opcode(op.name), isa_opcode=isa_opcode,
        perf_max=perf_max, ins=ins, outs=outs))


# --------------------------------------------------------------------------
# Device program
# --------------------------------------------------------------------------


def _build_program(pts_core=PTS_CORE, num_devices=NCORES):
    import concourse.bacc as bacc
    import concourse.bass as bass
    import concourse.tile as tile
    from concourse import mybir

    SEG, PSC = _register_dve_ops()
    ntiles = pts_core // TILE

    nc = bacc.Bacc("TRN2", target_bir_lowering=False, debug=False,
                   num_devices=num_devices)

    f32, f16 = mybir.dt.float32, mybir.dt.float16
    tgt_h = nc.dram_tensor("tgt_sh", [pts_core * KNBR, D], f16,
                           kind="ExternalInput").ap()
    qw_h = nc.dram_tensor("qw_sh", [pts_core, D], f16,
                          kind="ExternalInput").ap()
    wvt_h = nc.dram_tensor("wvt", [D, D], f16, kind="ExternalInput").ap()
    iden_h = nc.dram_tensor("iden", [D, D], f16, kind="ExternalInput").ap()
    out_h = nc.dram_tensor("out_sh", [pts_core, D], f16,
                           kind="ExternalOutput").ap()
    e2_h = nc.dram_tensor("e2_sh", [pts_core, 2 * KNBR], f16,
                          kind="ExternalOutput").ap()

    ACTF = mybir.ActivationFunctionType

    with tile.TileContext(nc) as tc:
        with (
            tc.tile_pool(name="consts", bufs=1) as consts,
            tc.tile_pool(name="qwp", bufs=1) as qwp,
            tc.tile_pool(name="outp", bufs=1) as outp,
            tc.tile_pool(name="tnp", bufs=8) as tnp,
            tc.tile_pool(name="cump", bufs=4) as cump,
            tc.tile_pool(name="sclp", bufs=4) as sclp,
            tc.tile_pool(name="smal", bufs=4) as smal,
            tc.tile_pool(name="ps", bufs=4, space="PSUM") as ps,
        ):
            tgt_v = tgt_h.rearrange("(n k) d -> n k d", k=KNBR)
            qw_v = qw_h.rearrange("(t p) d -> p t d", p=TILE)
            out_v = out_h.rearrange("(t p) d -> p t d", p=TILE)
            e2_v = e2_h.rearrange("(t p) e -> p t e", p=TILE)

            # segdot(0) critical path: tn(0) + first qw tile lead the queue
            qw_all = qwp.tile([TILE, ntiles, D], f16)
            tn_tiles = {}

            def load_tn(t):
                # alternate the big tgt streams across two DMA queues
                # (GpSimd's queue is otherwise idle) so transfers and
                # trigger issue run in parallel with the sync queue
                tn = tnp.tile([TILE, KNBR, D], f16, tag="tn")
                p0 = t * TILE
                eng = nc.gpsimd if t % 2 == 0 else nc.sync
                eng.dma_start(out=tn, in_=tgt_v[p0:p0 + TILE])
                tn_tiles[t] = tn

            load_tn(0)
            nc.sync.dma_start(out=qw_all[:, 0:1, :], in_=qw_v[:, 0:1, :])
            for t in range(1, min(6, ntiles)):
                load_tn(t)
            nc.sync.dma_start(out=qw_all[:, 1:4, :], in_=qw_v[:, 1:4, :])

            wvt_sb = consts.tile([D, D], f16)
            nc.sync.dma_start(out=wvt_sb, in_=wvt_h)
            iden_sb = consts.tile([D, D], f16)
            nc.sync.dma_start(out=iden_sb, in_=iden_h)
            for c in range(4, ntiles, 8):
                ce = min(c + 8, ntiles)
                nc.sync.dma_start(out=qw_all[:, c:ce, :], in_=qw_v[:, c:ce, :])

            out_all = outp.tile([TILE, ntiles, D], f16)
            e2_all = outp.tile([TILE, ntiles, 2 * KNBR], f16)

            cum_tiles = {}

            def emit_segdot(t):
                # scores: segmented dot products, page ends hold the result
                cum = cump.tile([TILE, KNBR, D], f16, tag="cum")
                qw_bk = bass.AP(tensor=qw_all.tensor,
                                offset=qw_all.offset + t * D,
                                ap=[qw_all.ap[0], [0, KNBR], [1, D]])
                _emit_custom(nc, SEG, out=cum, in0=tn_tiles[t], in1=qw_bk)
                cum_tiles[t] = cum

            emit_segdot(0)

            OGRP = 4  # output DMA granularity (tiles)
            for t in range(ntiles):
                # software pipeline: keep the DVE busy with segdot(t+1)
                # while ScalarE computes exp(t)
                if t + 1 < ntiles:
                    emit_segdot(t + 1)
                if t + 6 < ntiles:
                    load_tn(t + 6)
                tn = tn_tiles.pop(t)
                cum = cum_tiles.pop(t)

                # e2[p, k, 2] = exp(score[p, k]) twice — fp16 pairs so the
                # PAGESCALE latch consumes one aligned 4-byte pair per page
                ends = bass.AP(tensor=cum.tensor, offset=cum.offset + (D - 1),
                               ap=[cum.ap[0], [D, KNBR], [0, 2]])
                e2_w = bass.AP(tensor=e2_all.tensor,
                               offset=e2_all.offset + t * 2 * KNBR,
                               ap=[e2_all.ap[0], [2, KNBR], [1, 2]])
                nc.scalar.activation(e2_w, ends, ACTF.Exp)

                # scaled[p,k,:] = tn[p,k,:] * e[p,k]
                scaled = sclp.tile([TILE, KNBR, D], f16, tag="scl")
                e2_r = bass.AP(tensor=e2_all.tensor,
                               offset=e2_all.offset + t * 2 * KNBR,
                               ap=[e2_all.ap[0], [1, 2 * KNBR]])
                _emit_custom(nc, PSC, out=scaled, in0=tn, in1=e2_r)

                # ctxT[d, p] = sum_k scaled[p, k, d] via accumulating
                # identity-matmuls (transpose + reduce in one)
                ps_ctxT = ps.tile([D, TILE], f32, tag="psc")
                for k in range(KNBR):
                    nc.tensor.matmul(ps_ctxT, lhsT=scaled[:, k, :],
                                     rhs=iden_sb, start=(k == 0),
                                     stop=(k == KNBR - 1))
                ctxT_sb = smal.tile([D, TILE], f16, tag="ctxT")
                nc.scalar.copy(ctxT_sb, ps_ctxT)

                # out_raw = ctx @ Wv.T
                ps_out = ps.tile([TILE, D], f32, tag="pso")
                nc.tensor.matmul(ps_out, lhsT=ctxT_sb, rhs=wvt_sb,
                                 start=True, stop=True)
                nc.scalar.copy(out_all[:, t, :], ps_out)

                if t >= ntiles - 4:
                    # per-tile DMAs at the end so the tail only waits on the
                    # last tile's copy
                    nc.sync.dma_start(out=out_v[:, t:t + 1, :],
                                      in_=out_all[:, t:t + 1, :])
                    nc.sync.dma_start(out=e2_v[:, t:t + 1, :],
                                      in_=e2_all[:, t:t + 1, :])
                elif (t + 1) % OGRP == 0:
                    t0 = t + 1 - OGRP
                    nc.sync.dma_start(out=out_v[:, t0:t + 1, :],
                                      in_=out_all[:, t0:t + 1, :])
                    nc.sync.dma_start(out=e2_v[:, t0:t + 1, :],
                                      in_=e2_all[:, t0:t + 1, :])

    nc.compile()
    return nc


# --------------------------------------------------------------------------
# Host wrapper
# --------------------------------------------------------------------------


def kernel(src, tgt, Wq, Wk, Wv):
    from concourse.bass_utils import run_bass_kernel_spmd

    scale = 1.0 / math.sqrt(D)
    wqk = (Wq.astype(np.float64).T @ Wk.astype(np.float64)
           * scale).astype(np.float32)
    qw = (np.ascontiguousarray(src, dtype=np.float32).reshape(PTS_TOTAL, D)
          @ wqk).astype(np.float16)
    tn = np.ascontiguousarray(tgt, dtype=np.float32) \
        .reshape(PTS_TOTAL * KNBR, D).astype(np.float16)
    wvt = np.ascontiguousarray(Wv.astype(np.float32).T).astype(np.float16)
    iden = np.eye(D, dtype=np.float16)

    if "nc" not in _cached:
        _cached["nc"] = _build_program()
    nc = _cached["nc"]

    in_maps = []
    for c in range(NCORES):
        p0, p1 = c * PTS_CORE, (c + 1) * PTS_CORE
        in_maps.append({
            "tgt_sh": tn[p0 * KNBR:p1 * KNBR],
            "qw_sh": qw[p0:p1],
            "wvt": wvt,
            "iden": iden,
        })

    _cached["in_maps"] = in_maps
    res = run_bass_kernel_spmd(nc, in_maps, core_ids=list(range(NCORES)))
    out_raw = np.concatenate(
        [r["out_sh"].astype(np.float32) for r in res.results], axis=0)
    e2 = np.concatenate([r["e2_sh"] for r in res.results], axis=0)
    den = e2[:, 0::2].astype(np.float32).sum(axis=1, keepdims=True)
    out = out_raw / den
    return out.reshape(B, N_SEQ, D).astype(np.float32)


def __getattr__(name):
    if name == "_last_in_maps":
        return _cached.get("in_maps")
    raise AttributeError(name)


# revision 22
# speedup vs baseline: 1.0487x; 1.0487x over previous
"""Trainium2 Bass kernel for local sparse attention (k=16 neighbors).

Reference computation (b=4, n=8192, k=16, d=128):
    Q = src @ Wq.T ; K = tgt @ Wk.T ; V = tgt @ Wv.T
    scores = einsum('bnkd,bnd->bnk', K, Q) / sqrt(d)
    out = einsum('bnk,bnkd->bnd', softmax(scores), V)

Restructured so the 34-GFLOP K/V projections are never materialized:
    scores[n,k] = tgt[n,k,:] . qw[n,:],  qw = src @ (Wq.T Wk / sqrt(d))
    out[n,:]    = (sum_k e[n,k] * tgt[n,k,:]) @ Wv.T / sum_k e[n,k]

Device pipeline per 128-point tile (all tensors fp16, fp32 accumulation):
  1. SEGDOT_ANT   (custom DVE op, 2x_1p fp16 mode, hand-written uops):
     per-page dot products via a segmented multiply-cumsum whose
     accumulator RESETS at each 128-element page boundary; page-end
     positions hold scores[p, k]. Two fp16 MACs per lane-cycle.
  2. ScalarE exp on the 16 page-end values (read twice each ->
     duplicated pairs e2[p,k,2]).
  3. PAGESCALE_ANT (custom DVE op, 2x_1p): scaled[p,k,:] = tn[p,k,:] *
     e[p,k], with e held in swap flops and re-latched per page (src1 is
     consumed as one 4-byte pair per page -> e duplicated pairs).
  4. TensorE: 16 accumulating identity-matmuls produce
     ctxT[d, p] = sum_k scaled[p,k,d] directly in PSUM (transpose and
     reduction in one), then out_raw = ctxT.T @ Wv.T via one more matmul.
  5. fp16 out_raw + fp16 e2 DMA'd out; host computes den = sum_k e and
     divides.

Host does the tiny O(1/32 of flops) parts: qw projection, fp16 casts,
final divide. Sharding: data-parallel over (b*n) across 8 cores.
"""

import math

import numpy as np

B, N_SEQ, KNBR, D = 4, 8192, 16, 128
NCORES = 8
PTS_TOTAL = B * N_SEQ            # 32768
PTS_CORE = PTS_TOTAL // NCORES   # 4096
TILE = 128
NTILES = PTS_CORE // TILE        # 32

_cached = {}

# --------------------------------------------------------------------------
# Custom DVE ops (hand-written uop programs, 1x + 2x_1p variants)
# --------------------------------------------------------------------------


def _register_dve_ops():
    from concourse.dve_ops import (
        DveOp, OPS, CUSTOM_DVE_SPECS, _SUB_OPCODE_FOR_NAME,
        _CUSTOM_DVE_ROW_BASE,
    )
    have = {op.name: op for op in OPS}
    if "SEGDOT_ANT" in have:
        return have["SEGDOT_ANT"], have["PAGESCALE_ANT"]

    from dataclasses import dataclass
    from concourse.dve_spec import (
        Spec, Src0, Src1, C3, scan, AluOp, _spill_c3_to_src1,
    )
    from concourse.dve_uop import (
        DveOpSpec, UopConfig, AluInp, InpSel, OutSel, OutPath,
        Trigger, DelayInp, ENABLE,
    )

    Dl = [AluInp.PREV_DELAY_0, AluInp.PREV_DELAY_1, AluInp.PREV_DELAY_2,
          AluInp.PREV_DELAY_3, AluInp.PREV_DELAY_4, AluInp.PREV_DELAY_5]
    PREV, CURR, SWAP = (AluInp.PREV_ALU_OUT, AluInp.CURR_ALU_OUT,
                        AluInp.CURR_SWAP_OUT)
    T_STEADY = (Trigger.SRC_TENSOR_DONE, Trigger.SUB_DIM_DONE, Trigger.NONE)
    T_STEP = (Trigger.SRC_TENSOR_DONE, Trigger.SUB_DIM_DONE, Trigger.COUNT)
    T_SEED = (Trigger.COUNT, Trigger.NONE, Trigger.NONE)

    def mk(inputs, lanes):
        u = UopConfig()
        for lane_idx, sel in inputs:
            u.enable_input(sel, lane_idx + 1)
        for b in range(8):
            u.datapath_config[b].pass_through_alu()
            u.datapath_config[b].pass_through_delay(*lanes)
        return u

    def seed(u):
        u.trigger, u.repeat_count, u.next_uop = T_SEED, 1, (1, 0, 0)
        return u

    def steady(u, write=True):
        u.require_inp0 = u.require_inp1 = 1
        if write:
            u.enable_output(OutSel.ALU_OUT, OutPath.WR0_LO)
        u.trigger, u.next_uop = T_STEADY, (0, 2, 0)
        return u

    def step(u, write=True):
        u.require_inp0 = u.require_inp1 = 1
        if write:
            u.enable_output(OutSel.ALU_OUT, OutPath.WR0_LO)
        u.trigger, u.next_uop, u.repeat_count = T_STEP, (0, 2, 1), 1
        return u

    # ---- SEGDOT: reset-cumsum of in0*in1 over pages -----------------------
    def segdot_1x():
        INP = [(0, InpSel.SRC_0), (1, InpSel.SRC_1), (2, InpSel.ZERO)]

        def base():
            u = mk(INP, (0, 1, 2))
            u.datapath_config[0].enable_alu(AluOp.MULTIPLY, Dl[0], Dl[1])
            u.datapath_config[1].enable_alu(AluOp.ADD, CURR, PREV)
            return u

        s0 = base()
        s0.datapath_config[1].enable_alu(AluOp.BYPASS, Dl[2])   # acc <- 0
        s1 = steady(base())
        s2 = base()
        s2.datapath_config[1].enable_alu(AluOp.BYPASS, PREV)    # acc <- m
        return [seed(s0), s1, step(s2)]

    def segdot_2x():
        INP = [(0, InpSel.SRC_0), (1, InpSel.SRC_1),
               (2, InpSel.SRC_0_HI), (3, InpSel.SRC_1_HI), (4, InpSel.ZERO)]

        def base():
            u = mk(INP, (0, 1, 2, 3, 4))
            dp = u.datapath_config
            dp[0].enable_alu(AluOp.MULTIPLY, Dl[0], Dl[1])           # m0
            dp[1].enable_alu(AluOp.MULTIPLY, Dl[2], Dl[3])           # m1
            dp[1].enable_delay_from_src(DelayInp.PREV_ALU_OUT, 5)    # ch5<-m0
            for b in range(2, 8):
                dp[b].pass_through_delay(5)
            dp[2].enable_alu(AluOp.ADD, PREV, Dl[5])                 # t=m0+m1
            dp[3].enable_alu(AluOp.ADD, CURR, PREV)                  # acc+=t
            return u

        s0 = base()
        s0.datapath_config[3].enable_alu(AluOp.BYPASS, Dl[4])        # acc<-0
        s1 = steady(base())
        s1.enable_output(OutSel.ALU_OUT, OutPath.WR0_HI)  # pair-cumsum both halves
        s2 = base()
        s2.datapath_config[3].enable_alu(AluOp.BYPASS, PREV)         # acc<-t
        step(s2)
        s2.enable_output(OutSel.ALU_OUT, OutPath.WR0_HI)
        return [seed(s0), s1, s2]

    # ---- PAGESCALE: in0 * e[page], e latched per page from src1 ----------
    def pagescale_1x():
        INP = [(0, InpSel.SRC_0), (1, InpSel.SRC_1)]
        init = mk(INP, (0, 1))
        init.datapath_config[0].enable_alu(AluOp.BYPASS, Dl[1])
        init.datapath_config[0].swap_enable = ENABLE
        init.require_inp1 = 1
        seed(init)

        st = mk(INP, (0, 1))
        st.datapath_config[0].enable_alu(AluOp.MULTIPLY, Dl[0], SWAP)
        st.require_inp0 = 1
        st.enable_output(OutSel.ALU_OUT, OutPath.WR0_LO)
        st.trigger, st.next_uop = T_STEADY, (0, 2, 0)

        sp = mk(INP, (0, 1))
        sp.datapath_config[0].enable_alu(AluOp.BYPASS, Dl[1])
        sp.datapath_config[0].swap_enable = ENABLE
        sp.datapath_config[1].enable_alu(AluOp.MULTIPLY, Dl[0], PREV)
        step(sp)
        return [init, st, sp]

    def pagescale_2x():
        INP = [(0, InpSel.SRC_0), (1, InpSel.SRC_0_HI), (2, InpSel.SRC_1)]
        init = mk(INP, (0, 1, 2))
        init.datapath_config[0].enable_alu(AluOp.BYPASS, Dl[2])
        init.datapath_config[0].swap_enable = ENABLE
        init.datapath_config[1].enable_alu(AluOp.BYPASS, PREV)
        init.datapath_config[1].swap_enable = ENABLE
        init.require_inp1 = 1
        seed(init)

        st = mk(INP, (0, 1, 2))
        dp = st.datapath_config
        dp[0].enable_alu(AluOp.MULTIPLY, Dl[0], SWAP)                # r0
        dp[1].enable_alu(AluOp.MULTIPLY, Dl[1], SWAP)                # r1
        dp[1].enable_delay_from_src(DelayInp.PREV_ALU_OUT, 3)        # ch3<-r0
        for b in range(2, 8):
            dp[b].pass_through_delay(3)
        st.require_inp0 = 1
        st.enable_output(OutSel.DELAY_3, OutPath.WR0_LO)             # r0 even
        st.enable_output(OutSel.ALU_OUT, OutPath.WR0_HI)             # r1 odd
        st.trigger, st.next_uop = T_STEADY, (0, 2, 0)

        sp = mk(INP, (0, 1, 2))
        dp = sp.datapath_config
        dp[0].enable_alu(AluOp.BYPASS, Dl[2])                        # e_new
        dp[0].swap_enable = ENABLE
        dp[1].enable_alu(AluOp.BYPASS, PREV)
        dp[1].swap_enable = ENABLE
        dp[2].enable_alu(AluOp.MULTIPLY, Dl[0], PREV)                # r0
        dp[2].enable_delay_from_src(DelayInp.PREV_ALU_OUT, 3)        # ch3<-e
        dp[3].enable_alu(AluOp.MULTIPLY, Dl[1], Dl[3])               # r1
        dp[3].enable_delay_from_src(DelayInp.PREV_ALU_OUT, 4)        # ch4<-r0
        for b in range(3, 8):
            dp[b].pass_through_delay(3)
        for b in range(4, 8):
            dp[b].pass_through_delay(4)
        sp.require_inp0 = sp.require_inp1 = 1
        sp.enable_output(OutSel.DELAY_4, OutPath.WR0_LO)
        sp.enable_output(OutSel.ALU_OUT, OutPath.WR0_HI)
        sp.trigger, sp.next_uop, sp.repeat_count = T_STEP, (0, 2, 1), 1
        return [init, st, sp]

    @dataclass(frozen=True)
    class HandDveOp(DveOp):
        raw_v3: "DveOpSpec | None" = None

        def compile(self, ver):
            assert ver == "v3", f"hand-built op only has v3 uops, got {ver}"
            return self.raw_v3

    def segdot_ref(in0, in1, s0, s1, imm2):
        P, N = in0.shape[0], in0.shape[-1]
        S = int(np.prod(in0.shape[1:-1]))
        a = in0.reshape(P, S, N).astype(np.float32)
        b = np.asarray(in1, np.float32)
        b = (np.broadcast_to(b.reshape(P, 1, N), a.shape)
             if b.size == P * N else b.reshape(a.shape))
        return np.cumsum(a * b, axis=-1).reshape(in0.shape)

    def pagescale_ref(in0, in1, s0, s1, imm2):
        P, N = in0.shape[0], in0.shape[-1]
        S = int(np.prod(in0.shape[1:-1]))
        a = in0.reshape(P, S, N).astype(np.float32)
        e = np.asarray(in1, np.float32).reshape(P, -1)[:, ::2][:, :S]
        return (a * e[:, :, None]).reshape(in0.shape)

    ops = []
    for name, spec, u1, u2 in (
        ("SEGDOT_ANT",
         Spec(body=scan(AluOp.ADD, Src0 * Src1), reference=segdot_ref),
         segdot_1x(), segdot_2x()),
        ("PAGESCALE_ANT",
         Spec(body=Src0 * _spill_c3_to_src1(C3), reference=pagescale_ref),
         pagescale_1x(), pagescale_2x()),
    ):
        row = _CUSTOM_DVE_ROW_BASE + len(OPS)
        raw = DveOpSpec(name=name, opcode=row, uops=u1, uops_2x=u2,
                        perf_max=1, rd1_en=True)
        raw.validate("v3")
        op = HandDveOp(name=name, spec=spec, subdim=True,
                       uops_sha={"v3": raw.sha("v3")}, raw_v3=raw)
        OPS.append(op)
        _SUB_OPCODE_FOR_NAME[name] = row
        CUSTOM_DVE_SPECS[name] = spec
        ops.append(op)
    return ops[0], ops[1]


def _emit_custom(nc, op, *, out, in0, in1, perf_max=1):
    """Like nc.vector._custom_dve but with an explicit perf_max."""
    from concourse import bass_isa, mybir
    from concourse.dve_ops import get_dve_sub_opcode

    vec = nc.vector
    m = vec.bass.m
    if op.name not in m.ant_custom_dve_ops:
        m.ant_custom_dve_ops = sorted({*m.ant_custom_dve_ops, op.name})
    opt = not op.subdim
    in1_elementwise = len(in1.shape) > 2
    shape = (bass_isa.CustomDveShape.STT if in1_elementwise
             else bass_isa.CustomDveShape.TTSS)
    isa_opcode = vec.bass.isa.Opcode[
        f"NEURON_ISA_TPB_OPCODE_CUSTOM_DVE_ANT_{shape.slot()}"
    ].value
    zero = mybir.ImmediateValue(dtype=mybir.dt.float32, value=0.0)
    ins = [vec.lower_ap(in0, for_isa=True, opt=opt),
           vec.lower_ap(in1, for_isa=True, opt=opt), zero, zero]
    outs = [vec.lower_ap(out, for_isa=True, opt=opt)]
    return vec.add_instruction(bass_isa.InstCustomDveAnt(
        name=vec.bass.get_next_instruction_name(),
        op_name=op.name, rd1_en=True, subdim=0x02, imm2=0.0,
        shape=shape, row=get_dve_sub_opcode(op.name), isa_opcode=isa_opcode,
        perf_max=perf_max, ins=ins, outs=outs))


# --------------------------------------------------------------------------
# Device program
# --------------------------------------------------------------------------


def _build_program(pts_core=PTS_CORE, num_devices=NCORES):
    import concourse.bacc as bacc
    import concourse.bass as bass
    import concourse.tile as tile
    from concourse import mybir

    SEG, PSC = _register_dve_ops()
    ntiles = pts_core // TILE

    nc = bacc.Bacc("TRN2", target_bir_lowering=False, debug=False,
                   num_devices=num_devices)

    f32, f16 = mybir.dt.float32, mybir.dt.float16
    tgt_h = nc.dram_tensor("tgt_sh", [pts_core * KNBR, D], f16,
                           kind="ExternalInput").ap()
    qw_h = nc.dram_tensor("qw_sh", [pts_core, D], f16,
                          kind="ExternalInput").ap()
    wvt_h = nc.dram_tensor("wvt", [D, D], f16, kind="ExternalInput").ap()
    iden_h = nc.dram_tensor("iden", [D, D], f16, kind="ExternalInput").ap()
    out_h = nc.dram_tensor("out_sh", [pts_core, D], f16,
                           kind="ExternalOutput").ap()
    e2_h = nc.dram_tensor("e2_sh", [pts_core, 2 * KNBR], f16,
                          kind="ExternalOutput").ap()

    ACTF = mybir.ActivationFunctionType

    with tile.TileContext(nc) as tc:
        with (
            tc.tile_pool(name="consts", bufs=1) as consts,
            tc.tile_pool(name="qwp", bufs=1) as qwp,
            tc.tile_pool(name="outp", bufs=1) as outp,
            tc.tile_pool(name="tnp", bufs=8) as tnp,
            tc.tile_pool(name="cump", bufs=4) as cump,
            tc.tile_pool(name="sclp", bufs=4) as sclp,
            tc.tile_pool(name="smal", bufs=4) as smal,
            tc.tile_pool(name="ps", bufs=4, space="PSUM") as ps,
        ):
            tgt_v = tgt_h.rearrange("(n k) d -> n k d", k=KNBR)
            qw_v = qw_h.rearrange("(t p) d -> p t d", p=TILE)
            out_v = out_h.rearrange("(t p) d -> p t d", p=TILE)
            e2_v = e2_h.rearrange("(t p) e -> p t e", p=TILE)

            # segdot(0) critical path: tn(0) + first qw tile lead the queue
            qw_all = qwp.tile([TILE, ntiles, D], f16)
            tn_tiles = {}

            def load_tn(t):
                tn = tnp.tile([TILE, KNBR, D], f16, tag="tn")
                p0 = t * TILE
                nc.sync.dma_start(out=tn, in_=tgt_v[p0:p0 + TILE])
                tn_tiles[t] = tn

            load_tn(0)
            nc.sync.dma_start(out=qw_all[:, 0:1, :], in_=qw_v[:, 0:1, :])
            for t in range(1, min(6, ntiles)):
                load_tn(t)
            nc.sync.dma_start(out=qw_all[:, 1:4, :], in_=qw_v[:, 1:4, :])

            wvt_sb = consts.tile([D, D], f16)
            nc.sync.dma_start(out=wvt_sb, in_=wvt_h)
            iden_sb = consts.tile([D, D], f16)
            nc.sync.dma_start(out=iden_sb, in_=iden_h)
            for c in range(4, ntiles, 8):
                ce = min(c + 8, ntiles)
                nc.sync.dma_start(out=qw_all[:, c:ce, :], in_=qw_v[:, c:ce, :])

            out_all = outp.tile([TILE, ntiles, D], f16)
            e2_all = outp.tile([TILE, ntiles, 2 * KNBR], f16)

            cum_tiles = {}

            def emit_segdot(t):
                # scores: segmented dot products, page ends hold the result
                cum = cump.tile([TILE, KNBR, D], f16, tag="cum")
                qw_bk = bass.AP(tensor=qw_all.tensor,
                                offset=qw_all.offset + t * D,
                                ap=[qw_all.ap[0], [0, KNBR], [1, D]])
                _emit_custom(nc, SEG, out=cum, in0=tn_tiles[t], in1=qw_bk)
                cum_tiles[t] = cum

            emit_segdot(0)

            OGRP = 4  # output DMA granularity (tiles)
            for t in range(ntiles):
                # software pipeline: keep the DVE busy with segdot(t+1)
                # while ScalarE computes exp(t)
                if t + 1 < ntiles:
                    emit_segdot(t + 1)
                if t + 6 < ntiles:
                    load_tn(t + 6)
                tn = tn_tiles.pop(t)
                cum = cum_tiles.pop(t)

                # e2[p, k, 2] = exp(score[p, k]) twice — fp16 pairs so the
                # PAGESCALE latch consumes one aligned 4-byte pair per page
                ends = bass.AP(tensor=cum.tensor, offset=cum.offset + (D - 1),
                               ap=[cum.ap[0], [D, KNBR], [0, 2]])
                e2_w = bass.AP(tensor=e2_all.tensor,
                               offset=e2_all.offset + t * 2 * KNBR,
                               ap=[e2_all.ap[0], [2, KNBR], [1, 2]])
                nc.scalar.activation(e2_w, ends, ACTF.Exp)

                # scaled[p,k,:] = tn[p,k,:] * e[p,k]
                scaled = sclp.tile([TILE, KNBR, D], f16, tag="scl")
                e2_r = bass.AP(tensor=e2_all.tensor,
                               offset=e2_all.offset + t * 2 * KNBR,
                               ap=[e2_all.ap[0], [1, 2 * KNBR]])
                _emit_custom(nc, PSC, out=scaled, in0=tn, in1=e2_r)

                # ctxT[d, p] = sum_k scaled[p, k, d] via accumulating
                # identity-matmuls (transpose + reduce in one)
                ps_ctxT = ps.tile([D, TILE], f32, tag="psc")
                for k in range(KNBR):
                    nc.tensor.matmul(ps_ctxT, lhsT=scaled[:, k, :],
                                     rhs=iden_sb, start=(k == 0),
                                     stop=(k == KNBR - 1))
                ctxT_sb = smal.tile([D, TILE], f16, tag="ctxT")
                nc.scalar.copy(ctxT_sb, ps_ctxT)

                # out_raw = ctx @ Wv.T
                ps_out = ps.tile([TILE, D], f32, tag="pso")
                nc.tensor.matmul(ps_out, lhsT=ctxT_sb, rhs=wvt_sb,
                                 start=True, stop=True)
                nc.scalar.copy(out_all[:, t, :], ps_out)

                if t >= ntiles - 4:
                    # per-tile DMAs at the end so the tail only waits on the
                    # last tile's copy
                    nc.sync.dma_start(out=out_v[:, t:t + 1, :],
                                      in_=out_all[:, t:t + 1, :])
                    nc.sync.dma_start(out=e2_v[:, t:t + 1, :],
                                      in_=e2_all[:, t:t + 1, :])
                elif (t + 1) % OGRP == 0:
                    t0 = t + 1 - OGRP
                    nc.sync.dma_start(out=out_v[:, t0:t + 1, :],
                                      in_=out_all[:, t0:t + 1, :])
                    nc.sync.dma_start(out=e2_v[:, t0:t + 1, :],
                                      in_=e2_all[:, t0:t + 1, :])

    nc.compile()
    return nc


# --------------------------------------------------------------------------
# Host wrapper
# --------------------------------------------------------------------------


def kernel(src, tgt, Wq, Wk, Wv):
    from concourse.bass_utils import run_bass_kernel_spmd

    scale = 1.0 / math.sqrt(D)
    wqk = (Wq.astype(np.float64).T @ Wk.astype(np.float64)
           * scale).astype(np.float32)
    qw = (np.ascontiguousarray(src, dtype=np.float32).reshape(PTS_TOTAL, D)
          @ wqk).astype(np.float16)
    tn = np.ascontiguousarray(tgt, dtype=np.float32) \
        .reshape(PTS_TOTAL * KNBR, D).astype(np.float16)
    wvt = np.ascontiguousarray(Wv.astype(np.float32).T).astype(np.float16)
    iden = np.eye(D, dtype=np.float16)

    if "nc" not in _cached:
        _cached["nc"] = _build_program()
    nc = _cached["nc"]

    in_maps = []
    for c in range(NCORES):
        p0, p1 = c * PTS_CORE, (c + 1) * PTS_CORE
        in_maps.append({
            "tgt_sh": tn[p0 * KNBR:p1 * KNBR],
            "qw_sh": qw[p0:p1],
            "wvt": wvt,
            "iden": iden,
        })

    _cached["in_maps"] = in_maps
    res = run_bass_kernel_spmd(nc, in_maps, core_ids=list(range(NCORES)))
    out_raw = np.concatenate(
        [r["out_sh"].astype(np.float32) for r in res.results], axis=0)
    e2 = np.concatenate([r["e2_sh"] for r in res.results], axis=0)
    den = e2[:, 0::2].astype(np.float32).sum(axis=1, keepdims=True)
    out = out_raw / den
    return out.reshape(B, N_SEQ, D).astype(np.float32)


def __getattr__(name):
    if name == "_last_in_maps":
        return _cached.get("in_maps")
    raise AttributeError(name)
